# revision 15
# baseline (speedup 1.0000x reference)
"""AdjustedNonLocalBlock on 8 TRN2 NeuronCores (bf16 pipeline, dual-engine exp).

Math (per batch, N = H*W = 4096 positions):
    f = theta(x1)^T phi(x0);  P = softmax(f, axis=-1);
    y = P @ g(x0)^T;  out = W_w y^T + W_b + x0.

Reductions (as in the f32 baseline):
  - f[q,k] = x1[:,q]^T A x0[:,k] + t3[k] (+ per-q consts, dropped --
    softmax-invariant), A = theta_w^T phi_w, t3 = (phi_w^T theta_b)^T x0.
  - g's bias folds into b_out = W_w g_b + W_b; 1/Z applied between the
    attention and projection matmuls; Z via a ones-column in mm2's lhsT.

Precision plan (rel-err ~1e-3 << 2e-2 gate, validated in numpy + sim):
  - mm1 (S = U^T X1) and mm2 (Y += [g|1]^T E) in bf16.  (fp8 DoubleRow
    was measured on HW to give ZERO matmul speedup at K=128 -- DR doubles
    contraction per pass, not column rate -- so bf16 is the right dtype.)
  - exp is split per S tile between TWO engines: ScalarE does cols
    [0:SPLIT] with the table exp; DVE does [SPLIT:1024] with a
    Schraudolph fast-exp (i16 = a*(s+t3+40) + b, bitcast to bf16).
    Both produce e^(s+t3+40); the shared +40 shift keeps the i16
    affine positive and cancels per query in the softmax.

Dataflow per core (core i = (batch i//2, query half i%2), 2048 queries):
  All PSUM flows through one 3-slot [128,1024] pool (6 banks) + 2 Y
  banks.  ALL of U / gaug / t3 production is hoisted into the prologue,
  overlapped with the input DMA stream, so the main loop is pure
  mm1 -> exp -> mm2 at the PE floor.  Per (qp, kt): 2 bf16 mm1 ->
  S [128k,1024q]; ScalarE exp + DVE fast-exp -> shared e tile (bf16);
  2 bf16 mm2 into ya/yb [65,512].  Epilogue: Z row staged to SBUF
  (custom-DVE ops give garbage reading PSUM on HW), 1/Z via
  reciprocal_approx_fast, GPSIMD partition-broadcast, DVE normalize into
  yaug; projection (f32r) + residual add; qp0's projections run inside
  qp1 pinned behind a late mm2 (add_dep_helper) so the in-order PE
  never stalls on them.
"""

import numpy as np
import ml_dtypes

import concourse.bacc as bacc
import concourse.mybir as mybir
import concourse.tile as tile
from concourse.bass_utils import run_bass_kernel_spmd

B, C, CI = 4, 128, 64
H, W = 64, 64
N = H * W              # 4096
NCORES = 8
QH = N // 2            # 2048 queries per core
KT = N // 128          # 32 key tiles of 128
SPLIT = 608            # ScalarE exp cols per S tile (DVE takes the rest)

LN2 = float(np.log(2.0))
A_SCH = 128.0 / LN2            # Schraudolph slope for bf16-bitcast
SHIFT = 40.0                   # shared logit shift (cancels per query)
B_SCH = 127.0 * 128.0 - 3.5    # exponent bias minus sawtooth centering

F32 = mybir.dt.float32
F32R = mybir.dt.float32r
BF16 = mybir.dt.bfloat16
I16 = mybir.dt.int16

_CACHE = {}


def _f32(ap):
    return ap.bitcast(F32)


def _build():
    if "nc" in _CACHE:
        return _CACHE["nc"]

    nc = bacc.Bacc("TRN2", target_bir_lowering=False, debug=False,
                   num_devices=NCORES)
    x0_ext = nc.declare_dram_parameter("x0", [C, N], BF16, isOutput=False)
    x1_ext = nc.declare_dram_parameter("x1b", [C, QH], BF16, isOutput=False)
    res_ext = nc.declare_dram_parameter("res", [C, QH], F32, isOutput=False)
    at_ext = nc.declare_dram_parameter("a_t", [C, C], BF16, isOutput=False)
    gv_ext = nc.declare_dram_parameter("gv", [C, CI + 1], BF16, isOutput=False)
    wa_ext = nc.declare_dram_parameter("w_aug", [CI + 1, C], F32R,
                                       isOutput=False)
    out_ext = nc.declare_dram_parameter("out", [C, QH], F32, isOutput=True)

    AF = mybir.ActivationFunctionType
    MUL = mybir.AluOpType.mult
    ADD = mybir.AluOpType.add

    with tile.TileContext(nc, pool_alloc_mode="queue") as tc:
        with (
            tc.tile_pool(name="const", bufs=1) as constp,
            tc.tile_pool(name="data", bufs=1) as datap,
            tc.tile_pool(name="epool", bufs=4) as epool,
            tc.tile_pool(name="spool", bufs=3, space="PSUM") as spool,
            tc.tile_pool(name="ypool", bufs=2, space="PSUM") as ypool,
            tc.tile_pool(name="rzp", bufs=2) as rzp,
            tc.tile_pool(name="bcp", bufs=2) as bcp,
        ):
            # table preload: a tiny Exp warms the exp table set while
            # the input DMAs are still in flight
            scr = constp.tile([1, 2], F32)
            nc.vector.memset(scr[:], 1.0)
            nc.scalar.activation(scr[0:1, 1:2], scr[0:1, 0:1], AF.Exp)

            # PE warm-up: a short dummy burst during the DMA wait starts
            # the HAM clock ramp; the prologue's real U/gaug matmuls
            # finish it (a long burst here trips the activity throttle)
            wrm = constp.tile([C, 512], F32R)
            nc.vector.memset(_f32(wrm[:]), 0.0)
            wps = spool.tile([C, 1024], F32, tag="s")
            for _ in range(4):
                nc.tensor.matmul(wps[:, 0:512], wrm[:, 0:128], wrm[:],
                                 start=True, stop=True)

            # small inputs first, then x0 in 1024-col chunks gating the
            # prologue producers, then x1
            at_sb = constp.tile([C, C], BF16)
            nc.sync.dma_start(at_sb[:], at_ext[:])
            gv_sb = constp.tile([C, CI + 1], BF16)
            nc.sync.dma_start(gv_sb[:], gv_ext[:])
            x0_sb = datap.tile([C, N], BF16)
            x1_sb = datap.tile([C, QH], BF16)
            for c in range(4):
                nc.sync.dma_start(x0_sb[:, c * 1024:(c + 1) * 1024],
                                  x0_ext[:, c * 1024:(c + 1) * 1024])
            nc.sync.dma_start(x1_sb[:, 0:1024], x1_ext[:, 0:1024])
            nc.sync.dma_start(x1_sb[:, 1024:2048], x1_ext[:, 1024:2048])
            wa_sb = constp.tile([CI + 1, C], F32R)
            nc.sync.dma_start(wa_sb[:], wa_ext[:])

            ub_sb = datap.tile([C, KT, 128], BF16)     # U = A @ X0, bf16
            gaug_sb = datap.tile([C, KT, CI + 1], BF16)
            nc.vector.memset(gaug_sb[:], 1.0)          # preset Z ones cols
            t3p_sb = datap.tile([C, KT, 1], F32)   # t3 + SHIFT (exp bias)
            t3s_sb = datap.tile([C, KT, 1], F32)   # a*(t3+SHIFT) + B_SCH
            yaug_sb = datap.tile([CI + 1, QH], F32R)
            nc.vector.memset(_f32(yaug_sb)[CI:CI + 1, :], 1.0)
            res_sb = datap.tile([C, QH], F32)

            def emit_u_chunk(c):
                # 1024 keys; converts alternate ScalarE / DVE
                pu = spool.tile([C, 1024], F32, tag="s")
                nc.tensor.matmul(pu[:, 0:512], at_sb[:],
                                 x0_sb[:, c * 1024:c * 1024 + 512],
                                 start=True, stop=True)
                nc.tensor.matmul(pu[:, 512:1024], at_sb[:],
                                 x0_sb[:, c * 1024 + 512:(c + 1) * 1024],
                                 start=True, stop=True)
                dst = ub_sb[:, c * 8:(c + 1) * 8, :]
                if c % 2 == 0:
                    nc.scalar.activation(dst, pu[:], AF.Copy)
                else:
                    nc.vector.tensor_copy(dst, pu[:])

            def emit_gaug_batch(b):
                # 4 kt of [t3 | g^T] -> bf16 g stripes + fp32 bias cols
                pg = spool.tile([C, 4, CI + 1], F32, tag="s")
                for j in range(4):
                    kt = 4 * b + j
                    nc.tensor.matmul(pg[:, j, :],
                                     x0_sb[:, kt * 128:(kt + 1) * 128],
                                     gv_sb[:], start=True, stop=True)
                nc.vector.tensor_copy(gaug_sb[:, 4 * b:4 * b + 4, 0:CI],
                                      pg[:, :, 1:CI + 1])
                nc.scalar.activation(t3p_sb[:, 4 * b:4 * b + 4, :],
                                     pg[:, :, 0:1], AF.Copy, bias=SHIFT)
                nc.vector.tensor_scalar(t3s_sb[:, 4 * b:4 * b + 4, :],
                                        pg[:, :, 0:1], A_SCH,
                                        A_SCH * SHIFT + B_SCH, MUL, ADD)

            # full prologue hoist: all U chunks + gaug batches run during
            # the input DMA stream, keeping the main loop JIT-free
            for c in range(4):
                emit_u_chunk(c)
                emit_gaug_batch(2 * c)
                emit_gaug_batch(2 * c + 1)

            def emit_mm1(qp, kt):
                s = spool.tile([C, 1024], F32, tag="s")
                q0 = qp * 1024
                lhsT = ub_sb[:, kt, :]
                nc.tensor.matmul(s[:, 0:512], lhsT, x1_sb[:, q0:q0 + 512],
                                 start=True, stop=True)
                nc.tensor.matmul(s[:, 512:1024], lhsT,
                                 x1_sb[:, q0 + 512:q0 + 1024],
                                 start=True, stop=True)
                return s

            def emit_fronts(qp, ya, yb):
                # 1/Z -> broadcast across partitions -> normalize into
                # yaug; frees the Y banks for the next qp
                for i, Y in ((0, ya), (1, yb)):
                    qc = qp * 2 + i
                    # custom-DVE ops give garbage reading PSUM on HW --
                    # stage the Z row through SBUF first
                    zrow = rzp.tile([1, 512], F32, tag="zrow")
                    nc.vector.tensor_copy(zrow[:], Y[CI:CI + 1, :])
                    rz = rzp.tile([1, 512], F32)
                    nc.vector.reciprocal_approx_fast(rz[:], zrow[:])
                    bcs = bcp.tile([CI, 512], F32)
                    nc.gpsimd.partition_broadcast(bcs[:], rz[:], channels=CI)
                    nc.vector.tensor_mul(
                        yaug_sb[0:CI, qc * 512:(qc + 1) * 512],
                        Y[0:CI, :], bcs[:])

            def emit_back(qc, anchor=None):
                q0 = qc * 512
                pr = spool.tile([C, 1024], F32, tag="s")
                prj = nc.tensor.matmul(pr[:, 0:512], wa_sb[:],
                                       yaug_sb[:, q0:q0 + 512],
                                       start=True, stop=True)
                if anchor is not None:
                    # pin the projection behind a late matmul so the
                    # scheduler cannot hoist it into a stall
                    tile.add_dep_helper(prj.ins, anchor.ins, False,
                                        "defer epilogue proj")
                ot = epool.tile([C, 512], F32, tag="ot", bufs=2)
                nc.vector.tensor_add(ot[:], pr[:, 0:512],
                                     res_sb[:, q0:q0 + 512])
                nc.sync.dma_start(out_ext[:, q0:q0 + 512], ot[:])

            s_cur = emit_mm1(0, 0)
            prev_mm2 = None
            for qp in range(2):
                ya = ypool.tile([CI + 1, 512], F32, tag="y")
                yb = ypool.tile([CI + 1, 512], F32, tag="y")
                for kt in range(KT):
                    e = epool.tile([C, 1024], BF16)
                    nc.scalar.activation(e[:, 0:SPLIT], s_cur[:, 0:SPLIT],
                                         AF.Exp, bias=t3p_sb[:, kt, :])
                    nc.vector.tensor_scalar(e.bitcast(I16)[:, SPLIT:1024],
                                            s_cur[:, SPLIT:1024], A_SCH,
                                            t3s_sb[:, kt, :], MUL, ADD)
                    if qp == 0 and kt == 9:
                        nc.sync.dma_start(res_sb[:], res_ext[:])
                    if qp == 1:
                        # qp0's projections, far enough in that the
                        # normalized yaug halves are long ready
                        if kt == 10:
                            emit_back(0, anchor=prev_mm2)
                        elif kt == 12:
                            emit_back(1, anchor=prev_mm2)
                    if kt + 1 < KT:
                        s_nxt = emit_mm1(qp, kt + 1)
                    elif qp == 0:
                        s_nxt = emit_mm1(1, 0)
                    else:
                        s_nxt = None
                    st, sp = kt == 0, kt == KT - 1
                    glhs = gaug_sb[:, kt, :]
                    prev_mm2 = nc.tensor.matmul(ya[:], glhs, e[:, 0:512],
                                                start=st, stop=sp)
                    nc.tensor.matmul(yb[:], glhs, e[:, 512:1024],
                                     start=st, stop=sp)
                    s_cur = s_nxt
                emit_fronts(qp, ya, yb)

            emit_back(2)
            emit_back(3)

    nc.compile()
    _CACHE["nc"] = nc
    return nc


def _prep_in_maps(inputs):
    bf = ml_dtypes.bfloat16
    x0 = np.ascontiguousarray(np.asarray(inputs["x0"], np.float32)
                              ).reshape(B, C, N)
    x1 = np.ascontiguousarray(np.asarray(inputs["x1"], np.float32)
                              ).reshape(B, C, N)
    g_w = np.asarray(inputs["g_w"], np.float32)
    g_b = np.asarray(inputs["g_b"], np.float32)
    theta_w = np.asarray(inputs["theta_w"], np.float32)
    theta_b = np.asarray(inputs["theta_b"], np.float32)
    phi_w = np.asarray(inputs["phi_w"], np.float32)
    W_w = np.asarray(inputs["W_w"], np.float32)
    W_b = np.asarray(inputs["W_b"], np.float32)

    a_t = np.ascontiguousarray((phi_w.T @ theta_w).astype(bf))   # [C, C]
    v = phi_w.T @ theta_b                                        # [C]
    gv = np.ascontiguousarray(np.concatenate(
        [v[:, None], g_w.T], axis=1).astype(bf))                 # [C, 65]
    b_out = W_w @ g_b + W_b                                      # [C]
    w_aug = np.ascontiguousarray(
        np.concatenate([W_w.T, b_out[None, :]], axis=0))         # [65, C]

    x0_bf = x0.astype(bf)
    x1_bf = x1.astype(bf)

    in_maps = []
    for core in range(NCORES):
        b, hh = core // 2, core % 2
        in_maps.append({
            "x0": x0_bf[b],
            "x1b": np.ascontiguousarray(x1_bf[b][:, hh * QH:(hh + 1) * QH]),
            "res": np.ascontiguousarray(x0[b][:, hh * QH:(hh + 1) * QH]),
            "a_t": a_t,
            "gv": gv,
            "w_aug": w_aug,
        })
    return in_maps


def _run(inputs, trace=False):
    nc = _build()
    in_maps = _prep_in_maps(inputs)
    res = run_bass_kernel_spmd(nc, in_maps, core_ids=list(range(NCORES)),
                               trace=trace)
    out = np.empty((B, C, N), np.float32)
    for core in range(NCORES):
        b, hh = core // 2, core % 2
        out[b][:, hh * QH:(hh + 1) * QH] = res.results[core]["out"]
    return out.reshape(B, C, H, W), res


def kernel(**inputs) -> np.ndarray:
    out, _ = _run(inputs, trace=False)
    return out


# revision 16
# speedup vs baseline: 1.1625x; 1.1625x over previous
"""AdjustedNonLocalBlock on 8 TRN2 NeuronCores (bf16 pipeline, dual-engine exp).

Math (per batch, N = H*W = 4096 positions):
    f = theta(x1)^T phi(x0);  P = softmax(f, axis=-1);
    y = P @ g(x0)^T;  out = W_w y^T + W_b + x0.

Reductions (as in the f32 baseline):
  - f[q,k] = x1[:,q]^T A x0[:,k] + t3[k] (+ per-q consts, dropped --
    softmax-invariant), A = theta_w^T phi_w, t3 = (phi_w^T theta_b)^T x0.
  - g's bias folds into b_out = W_w g_b + W_b; 1/Z applied between the
    attention and projection matmuls; Z via a ones-column in mm2's lhsT.

Precision plan (rel-err ~1e-3 << 2e-2 gate, validated in numpy + sim):
  - mm1 (S = U^T X1) and mm2 (Y += [g|1]^T E) in bf16.  (fp8 DoubleRow
    was measured on HW to give ZERO matmul speedup at K=128 -- DR doubles
    contraction per pass, not column rate -- so bf16 is the right dtype.)
  - exp is split per S tile between TWO engines: ScalarE does cols
    [0:SPLIT] with the table exp; DVE does [SPLIT:1024] with a
    Schraudolph fast-exp (i16 = a*(s+t3+40) + b, bitcast to bf16).
    Both produce e^(s+t3+40); the shared +40 shift keeps the i16
    affine positive and cancels per query in the softmax.

Dataflow per core (core i = (batch i//2, query half i%2), 2048 queries):
  All PSUM flows through one 3-slot [128,1024] pool (6 banks) + 2 Y
  banks.  ALL of U / gaug / t3 production is hoisted into the prologue,
  overlapped with the input DMA stream, so the main loop is pure
  mm1 -> exp -> mm2 at the PE floor.  Per (qp, kt): 2 bf16 mm1 ->
  S [128k,1024q]; ScalarE exp + DVE fast-exp -> shared e tile (bf16);
  2 bf16 mm2 into ya/yb [65,512].  Epilogue: Z row staged to SBUF
  (custom-DVE ops give garbage reading PSUM on HW), 1/Z via
  reciprocal_approx_fast, GPSIMD partition-broadcast, DVE normalize into
  yaug; projection (f32r) + residual add; qp0's projections run inside
  qp1 pinned behind a late mm2 (add_dep_helper) so the in-order PE
  never stalls on them.
"""

import numpy as np
import ml_dtypes

import concourse.bacc as bacc
import concourse.mybir as mybir
import concourse.tile as tile
from concourse.bass_utils import run_bass_kernel_spmd

B, C, CI = 4, 128, 64
H, W = 64, 64
N = H * W              # 4096
NCORES = 8
QH = N // 2            # 2048 queries per core
KT = N // 128          # 32 key tiles of 128
SPLIT = 608            # ScalarE exp cols per S tile (DVE takes the rest)

LN2 = float(np.log(2.0))
A_SCH = 128.0 / LN2            # Schraudolph slope for bf16-bitcast
SHIFT = 40.0                   # shared logit shift (cancels per query)
B_SCH = 127.0 * 128.0 - 3.5    # exponent bias minus sawtooth centering

F32 = mybir.dt.float32
F32R = mybir.dt.float32r
BF16 = mybir.dt.bfloat16
F8 = mybir.dt.float8e4
I16 = mybir.dt.int16

_CACHE = {}


def _f32(ap):
    return ap.bitcast(F32)


def _build():
    if "nc" in _CACHE:
        return _CACHE["nc"]

    nc = bacc.Bacc("TRN2", target_bir_lowering=False, debug=False,
                   num_devices=NCORES)
    x0_ext = nc.declare_dram_parameter("x0", [C, N], BF16, isOutput=False)
    x1_ext = nc.declare_dram_parameter("x1dr", [C, 2, QH], F8, isOutput=False)
    res_ext = nc.declare_dram_parameter("res", [C, QH], F32, isOutput=False)
    at_ext = nc.declare_dram_parameter("a_t", [C, C], BF16, isOutput=False)
    gv_ext = nc.declare_dram_parameter("gv", [C, CI + 1], BF16, isOutput=False)
    wa_ext = nc.declare_dram_parameter("w_aug", [CI + 1, C], F32R,
                                       isOutput=False)
    out_ext = nc.declare_dram_parameter("out", [C, QH], F32, isOutput=True)

    AF = mybir.ActivationFunctionType
    DR = mybir.MatmulPerfMode.DoubleRow
    MUL = mybir.AluOpType.mult
    ADD = mybir.AluOpType.add

    with tile.TileContext(nc, pool_alloc_mode="queue") as tc:
        with (
            tc.tile_pool(name="const", bufs=1) as constp,
            tc.tile_pool(name="data", bufs=1) as datap,
            tc.tile_pool(name="epool", bufs=4) as epool,
            tc.tile_pool(name="spool", bufs=3, space="PSUM") as spool,
            tc.tile_pool(name="ypool", bufs=2, space="PSUM") as ypool,
            tc.tile_pool(name="rzp", bufs=2) as rzp,
            tc.tile_pool(name="bcp", bufs=2) as bcp,
        ):
            # table preload: a tiny Exp warms the exp table set while
            # the input DMAs are still in flight
            scr = constp.tile([1, 2], F32)
            nc.vector.memset(scr[:], 1.0)
            nc.scalar.activation(scr[0:1, 1:2], scr[0:1, 0:1], AF.Exp)

            # PE warm-up: a short dummy burst during the DMA wait starts
            # the HAM clock ramp; the prologue's real U/gaug matmuls
            # finish it (a long burst here trips the activity throttle)
            wrm = constp.tile([C, 512], F32R)
            nc.vector.memset(_f32(wrm[:]), 0.0)
            wps = spool.tile([C, 1024], F32, tag="s")
            for _ in range(4):
                nc.tensor.matmul(wps[:, 0:512], wrm[:, 0:128], wrm[:],
                                 start=True, stop=True)

            # small inputs first, then x0 in 1024-col chunks gating the
            # prologue producers, then x1
            at_sb = constp.tile([C, C], BF16)
            nc.sync.dma_start(at_sb[:], at_ext[:])
            gv_sb = constp.tile([C, CI + 1], BF16)
            nc.sync.dma_start(gv_sb[:], gv_ext[:])
            x0_sb = datap.tile([C, N], BF16)
            x1_sb = datap.tile([C, 2, QH], F8)
            for c in range(4):
                nc.sync.dma_start(x0_sb[:, c * 1024:(c + 1) * 1024],
                                  x0_ext[:, c * 1024:(c + 1) * 1024])
            nc.sync.dma_start(x1_sb[:, :, 0:1024], x1_ext[:, :, 0:1024])
            nc.sync.dma_start(x1_sb[:, :, 1024:2048], x1_ext[:, :, 1024:2048])
            wa_sb = constp.tile([CI + 1, C], F32R)
            nc.sync.dma_start(wa_sb[:], wa_ext[:])

            # U8: per kt a fp8 stripe; DR's second k-plane is stripe
            # kt+1 (contracts against X1's zero plane; only stripe KT
            # needs explicit zeros, for kt=31)
            u8_sb = datap.tile([C, KT + 1, 128], F8)
            nc.vector.memset(u8_sb[:, KT, :], 0.0)
            gaug_sb = datap.tile([C, KT, CI + 1], BF16)
            nc.vector.memset(gaug_sb[:], 1.0)          # preset Z ones cols
            t3p_sb = datap.tile([C, KT, 1], F32)   # t3 + SHIFT (exp bias)
            t3s_sb = datap.tile([C, KT, 1], F32)   # a*(t3+SHIFT) + B_SCH
            yaug_sb = datap.tile([CI + 1, QH], F32R)
            nc.vector.memset(_f32(yaug_sb)[CI:CI + 1, :], 1.0)
            res_sb = datap.tile([C, QH], F32)

            def emit_u_chunk(c):
                # 1024 keys; converts alternate ScalarE / DVE
                pu = spool.tile([C, 1024], F32, tag="s")
                nc.tensor.matmul(pu[:, 0:512], at_sb[:],
                                 x0_sb[:, c * 1024:c * 1024 + 512],
                                 start=True, stop=True)
                nc.tensor.matmul(pu[:, 512:1024], at_sb[:],
                                 x0_sb[:, c * 1024 + 512:(c + 1) * 1024],
                                 start=True, stop=True)
                dst = u8_sb[:, c * 8:(c + 1) * 8, :]
                if c % 2 == 0:
                    nc.scalar.activation(dst, pu[:], AF.Copy)
                else:
                    nc.vector.tensor_copy(dst, pu[:])

            def emit_gaug_batch(b):
                # 4 kt of [t3 | g^T] -> bf16 g stripes + fp32 bias cols
                pg = spool.tile([C, 4, CI + 1], F32, tag="s")
                for j in range(4):
                    kt = 4 * b + j
                    nc.tensor.matmul(pg[:, j, :],
                                     x0_sb[:, kt * 128:(kt + 1) * 128],
                                     gv_sb[:], start=True, stop=True)
                nc.vector.tensor_copy(gaug_sb[:, 4 * b:4 * b + 4, 0:CI],
                                      pg[:, :, 1:CI + 1])
                nc.scalar.activation(t3p_sb[:, 4 * b:4 * b + 4, :],
                                     pg[:, :, 0:1], AF.Copy, bias=SHIFT)
                nc.vector.tensor_scalar(t3s_sb[:, 4 * b:4 * b + 4, :],
                                        pg[:, :, 0:1], A_SCH,
                                        A_SCH * SHIFT + B_SCH, MUL, ADD)

            # full prologue hoist: all U chunks + gaug batches run during
            # the input DMA stream, keeping the main loop JIT-free
            for c in range(4):
                emit_u_chunk(c)
                emit_gaug_batch(2 * c)
                emit_gaug_batch(2 * c + 1)

            def emit_mm1(qp, kt):
                s = spool.tile([C, 1024], F32, tag="s")
                q0 = qp * 1024
                lhsT = u8_sb[:, kt:kt + 2, :]
                nc.tensor.matmul(s[:, 0:512], lhsT,
                                 x1_sb[:, :, q0:q0 + 512],
                                 start=True, stop=True, perf_mode=DR)
                nc.tensor.matmul(s[:, 512:1024], lhsT,
                                 x1_sb[:, :, q0 + 512:q0 + 1024],
                                 start=True, stop=True, perf_mode=DR)
                return s

            def emit_fronts(qp, ya, yb):
                # 1/Z -> broadcast across partitions -> normalize into
                # yaug; frees the Y banks for the next qp
                for i, Y in ((0, ya), (1, yb)):
                    qc = qp * 2 + i
                    # custom-DVE ops give garbage reading PSUM on HW --
                    # stage the Z row through SBUF first
                    zrow = rzp.tile([1, 512], F32, tag="zrow")
                    nc.vector.tensor_copy(zrow[:], Y[CI:CI + 1, :])
                    rz = rzp.tile([1, 512], F32)
                    nc.vector.reciprocal_approx_fast(rz[:], zrow[:])
                    bcs = bcp.tile([CI, 512], F32)
                    nc.gpsimd.partition_broadcast(bcs[:], rz[:], channels=CI)
                    nc.vector.tensor_mul(
                        yaug_sb[0:CI, qc * 512:(qc + 1) * 512],
                        Y[0:CI, :], bcs[:])

            def emit_back(qc, anchor=None):
                q0 = qc * 512
                pr = spool.tile([C, 1024], F32, tag="s")
                prj = nc.tensor.matmul(pr[:, 0:512], wa_sb[:],
                                       yaug_sb[:, q0:q0 + 512],
                                       start=True, stop=True)
                if anchor is not None:
                    # pin the projection behind a late matmul so the
                    # scheduler cannot hoist it into a stall
                    tile.add_dep_helper(prj.ins, anchor.ins, False,
                                        "defer epilogue proj")
                ot = epool.tile([C, 512], F32, tag="ot", bufs=2)
                nc.vector.tensor_add(ot[:], pr[:, 0:512],
                                     res_sb[:, q0:q0 + 512])
                nc.sync.dma_start(out_ext[:, q0:q0 + 512], ot[:])

            s_cur = emit_mm1(0, 0)
            prev_mm2 = None
            for qp in range(2):
                ya = ypool.tile([CI + 1, 512], F32, tag="y")
                yb = ypool.tile([CI + 1, 512], F32, tag="y")
                for kt in range(KT):
                    e = epool.tile([C, 1024], BF16)
                    nc.scalar.activation(e[:, 0:SPLIT], s_cur[:, 0:SPLIT],
                                         AF.Exp, bias=t3p_sb[:, kt, :])
                    nc.vector.tensor_scalar(e.bitcast(I16)[:, SPLIT:1024],
                                            s_cur[:, SPLIT:1024], A_SCH,
                                            t3s_sb[:, kt, :], MUL, ADD)
                    if qp == 0 and kt == 9:
                        nc.sync.dma_start(res_sb[:], res_ext[:])
                    if qp == 1:
                        # qp0's projections, far enough in that the
                        # normalized yaug halves are long ready
                        if kt == 10:
                            emit_back(0, anchor=prev_mm2)
                        elif kt == 12:
                            emit_back(1, anchor=prev_mm2)
                    if kt + 1 < KT:
                        s_nxt = emit_mm1(qp, kt + 1)
                    elif qp == 0:
                        s_nxt = emit_mm1(1, 0)
                    else:
                        s_nxt = None
                    st, sp = kt == 0, kt == KT - 1
                    glhs = gaug_sb[:, kt, :]
                    prev_mm2 = nc.tensor.matmul(ya[:], glhs, e[:, 0:512],
                                                start=st, stop=sp)
                    nc.tensor.matmul(yb[:], glhs, e[:, 512:1024],
                                     start=st, stop=sp)
                    s_cur = s_nxt
                emit_fronts(qp, ya, yb)

            emit_back(2)
            emit_back(3)

    nc.compile()
    _CACHE["nc"] = nc
    return nc


def _prep_in_maps(inputs):
    bf = ml_dtypes.bfloat16
    x0 = np.ascontiguousarray(np.asarray(inputs["x0"], np.float32)
                              ).reshape(B, C, N)
    x1 = np.ascontiguousarray(np.asarray(inputs["x1"], np.float32)
                              ).reshape(B, C, N)
    g_w = np.asarray(inputs["g_w"], np.float32)
    g_b = np.asarray(inputs["g_b"], np.float32)
    theta_w = np.asarray(inputs["theta_w"], np.float32)
    theta_b = np.asarray(inputs["theta_b"], np.float32)
    phi_w = np.asarray(inputs["phi_w"], np.float32)
    W_w = np.asarray(inputs["W_w"], np.float32)
    W_b = np.asarray(inputs["W_b"], np.float32)

    a_t = np.ascontiguousarray((phi_w.T @ theta_w).astype(bf))   # [C, C]
    v = phi_w.T @ theta_b                                        # [C]
    gv = np.ascontiguousarray(np.concatenate(
        [v[:, None], g_w.T], axis=1).astype(bf))                 # [C, 65]
    b_out = W_w @ g_b + W_b                                      # [C]
    w_aug = np.ascontiguousarray(
        np.concatenate([W_w.T, b_out[None, :]], axis=0))         # [65, C]

    x0_bf = x0.astype(bf)
    f8 = ml_dtypes.float8_e4m3

    in_maps = []
    for core in range(NCORES):
        b, hh = core // 2, core % 2
        x1dr = np.zeros((C, 2, QH), f8)
        x1dr[:, 0, :] = x1[b][:, hh * QH:(hh + 1) * QH].astype(f8)
        in_maps.append({
            "x0": x0_bf[b],
            "x1dr": x1dr,
            "res": np.ascontiguousarray(x0[b][:, hh * QH:(hh + 1) * QH]),
            "a_t": a_t,
            "gv": gv,
            "w_aug": w_aug,
        })
    return in_maps


def _run(inputs, trace=False):
    nc = _build()
    in_maps = _prep_in_maps(inputs)
    res = run_bass_kernel_spmd(nc, in_maps, core_ids=list(range(NCORES)),
                               trace=trace)
    out = np.empty((B, C, N), np.float32)
    for core in range(NCORES):
        b, hh = core // 2, core % 2
        out[b][:, hh * QH:(hh + 1) * QH] = res.results[core]["out"]
    return out.reshape(B, C, H, W), res


def kernel(**inputs) -> np.ndarray:
    out, _ = _run(inputs, trace=False)
    return out
